# revision 3
# baseline (speedup 1.0000x reference)
"""ConvTasNet-style decoder kernel for Trainium2 (8 NeuronCores).

Computation (per batch m, channel c):
    s[n,k]    = mixture_w[n,k] * est_mask[c,n,k]          n=512, k=16000
    frames    = W @ s                                      [16, 16000]
    out[8q+r] = frames[r, q] + frames[r+8, q-1]            overlap-add, hop 8

Sharding: 8 cores = (m in 0..3) x (k-half in 0..1). Each core handles both
channels of one (m, k-half) so mixture_w is read once, not twice. The
8-sample overlap-add seam between the two k-halves is summed on the host.

v2 layout (sim 83.6us -> target ~56us): est_mask ships as uint8
(q = round(mask*255); the 1/255 de-quant scale is folded into W on the
host), halving the dominant HBM stream. mixture_w stays fp16. The DVE
runs mixed-dtype u8*f16 multiplies at 1 elem/cycle (the 2x mode needs
all-2-byte operands), so the multiply no longer fits on the DVE alone:
channel 0 and the head columns of channel 1 multiply on the DVE, the
tail columns of channel 1 on GpSimd (Pool). The overlap-add runs in
"mm" mode for both channels (8 shifted matmuls accumulate straight into
PSUM [8, q]); the scalar engine stages PSUM->SBUF for the output DMA.
GpSimd cannot touch PSUM (BIR verifier), so Pool only ever writes SBUF.
All chunks are >=511 columns so every u8 mask DMA descriptor is >=512B
(smaller descriptors transfer at half rate).

Zero-pad columns at the k-range edges make the boundary terms exact; the
pads are zeroed in the st tiles on the same engine that owns those
columns (zeroing pad columns of the DMA-target tiles instead was
sporadically corrupted on hardware - never share the fringe of a
DMA-written region with another engine's writes).

The device writes output in [C, 8, QH] layout (contiguous DMA); the host
permutes to the final interleaved [C, TH] layout while unsharding.
"""

import sys

sys.path.insert(0, "/opt/trn_rl_repo")

import numpy as np

M, C, N, K, L = 4, 2, 512, 16000, 16
HOP = L // 2            # 8
KH = K // 2             # 8000 frames per core
QH = KH + 1             # 8001 output blocks per core
TH = QH * HOP           # 64008 samples per core per channel
T = (K - 1) * HOP + L   # 128008 full output samples
SUB = 512               # PSUM accumulation group width (one bank of fp32)
NB = N // 128           # 4 contraction subtiles
IN_DT = "float16"        # dtype for mw/w device transfer
MASK_DT = "uint8"        # est_mask ships quantized; scale folded into W
QSTEP = SUB
# Chunks >= 511 keep u8 mask descriptors (nk = ck+1 bytes) >= 512B (full
# DMA rate); the small head/tail chunks trade a little descriptor-rate
# penalty for a shorter pipeline fill and drain.
CHUNKS = [256, 1024, 1536, 1536, 1536, 1024, 576, 256, 256]  # sum = KH
WTILE = -(-(max(CHUNKS) + 2) // 16) * 16  # chunk + shift col, 32B-aligned rows
# Column where channel-1 multiplies switch from DVE to Pool. Pool handles
# cols [POOL_COL0, ns+1) of both b-halves of channel 1 (~35% of elements).
POOL_COL0 = 160
REPEAT = 1               # >1: wrap body in an on-device loop (timing only)
IN_BUFS = 3              # mw/mask chunk double-buffer depth
SPLIT_B = 2              # split input DMAs/muls into b-groups for finer overlap

_CACHE = {}


def _build_nc():
    import concourse.tile as tile
    from concourse import bacc, mybir

    f32 = mybir.dt.float32
    din = getattr(mybir.dt, IN_DT)
    dmask = getattr(mybir.dt, MASK_DT)

    nc = bacc.Bacc("TRN2", target_bir_lowering=False, debug=False, num_devices=8)
    mw_d = nc.dram_tensor("mw", [N, KH], din, kind="ExternalInput")
    mask_d = nc.dram_tensor("mask", [C * N, KH], dmask, kind="ExternalInput")
    w_d = nc.dram_tensor("w", [N, L], din, kind="ExternalInput")  # W^T / 255
    out_d = nc.dram_tensor("out", [C, HOP, QH], f32, kind="ExternalOutput")

    mw_v = mw_d.ap().rearrange("(b p) k -> p b k", p=128)      # [128, 4, KH]
    mask_v = mask_d.ap().rearrange("(b p) k -> p b k", p=128)  # [128, 8, KH]
    w_v = w_d.ap().rearrange("(b p) l -> p b l", p=128)        # [128, 4, L]

    with tile.TileContext(nc) as tc:
        with (
            tc.tile_pool(name="const", bufs=1) as cpool,
            tc.tile_pool(name="mwp", bufs=IN_BUFS) as mwp,
            tc.tile_pool(name="maskp", bufs=IN_BUFS) as maskp,
            tc.tile_pool(name="sp", bufs=10) as sp,
            tc.tile_pool(name="obp", bufs=4) as obp,
            tc.tile_pool(name="pop", bufs=8, space="PSUM") as pop,
        ):
            wt = cpool.tile([128, NB, L], din)
            nc.scalar.dma_start(wt[:], w_v)

            def chunk_body(ci, ck, k0):
                # chunk produces output columns q in [k0, k0 + nq);
                # input tile col j holds frame k = k0 - 1 + j
                k_lo = max(k0 - 1, 0)
                k_hi = min(k0 + ck, KH)
                nk = k_hi - k_lo
                j0 = k_lo - (k0 - 1)      # 1 for first chunk, else 0
                last = ci == len(CHUNKS) - 1
                nq = (QH - k0) if last else ck

                nh = SPLIT_B if SPLIT_B else 1
                hb = NB // nh  # b-rows of mw per half
                mwts, maskts = [], {}
                for hi in range(nh):
                    mwt = mwp.tile([128, hb, WTILE], din, tag=f"mwt{hi}")
                    nc.sync.dma_start(
                        mwt[:, :, j0 : j0 + nk],
                        mw_v[:, hi * hb : (hi + 1) * hb, k_lo:k_hi],
                    )
                    mwts.append(mwt)
                    # one tile per (b-half, channel): exactly one DMA writer
                    # per tile keeps the dependency tracking unambiguous
                    for cc in range(C):
                        mt = maskp.tile([128, hb, WTILE], dmask, tag=f"maskt{hi}{cc}")
                        nc.sync.dma_start(
                            mt[:, :, j0 : j0 + nk],
                            mask_v[:, cc * NB + hi * hb : cc * NB + (hi + 1) * hb, k_lo:k_hi],
                        )
                        maskts[(hi, cc)] = mt

                for cc in range(C):
                    ob = obp.tile([HOP, max(CHUNKS) + 1], f32, tag="ob")
                    for qs in range(0, nq, QSTEP):
                        ns = min(QSTEP, nq - qs)
                        # st col t holds frame k = k0 - 1 + qs + t; data cols
                        # of the input tiles are [j0, j0 + nk)
                        t0 = max(j0 - qs, 0)
                        t1 = min(j0 + nk - qs, ns + 1)
                        sts = []
                        for hi in range(nh):
                            sth = sp.tile([128, hb, SUB + 2], din, tag=f"st{hi}")
                            # engine -> column ranges of [0, ns+1)
                            splits = (
                                [(nc.vector, 0, POOL_COL0), (nc.gpsimd, POOL_COL0, ns + 1)]
                                if cc == 1
                                else [(nc.vector, 0, ns + 1)]
                            )
                            for eng, c_lo, c_hi in splits:
                                d0 = max(c_lo, t0)
                                d1 = min(c_hi, t1)
                                if d1 > d0:
                                    eng.tensor_mul(
                                        sth[:, :, d0:d1],
                                        maskts[(hi, cc)][:, :, qs + d0 : qs + d1],
                                        mwts[hi][:, :, qs + d0 : qs + d1],
                                    )
                                if d0 > c_lo:
                                    eng.memset(sth[:, :, c_lo:d0], 0.0)
                                if c_hi > max(d1, d0):
                                    eng.memset(sth[:, :, max(d1, d0) : c_hi], 0.0)
                            sts.append(sth)

                        def st_slice(ni, a, b):
                            return sts[ni // hb][:, ni % hb, a:b]

                        po = pop.tile([HOP, SUB], f32, tag="po")
                        for h in range(2):
                            # top (h=0) reads k=q (col qs+1+); bottom k=q-1
                            for ni in range(NB):
                                nc.tensor.matmul(
                                    po[:, 0:ns],
                                    wt[:, ni, h * HOP : (h + 1) * HOP],
                                    st_slice(ni, 1 - h, 1 - h + ns),
                                    start=(h == 0 and ni == 0),
                                    stop=(h == 1 and ni == NB - 1),
                                )
                        nc.scalar.copy(ob[:, qs : qs + ns], po[:, 0:ns])
                    nc.scalar.dma_start(out_d.ap()[cc, :, k0 : k0 + nq], ob[:, 0:nq])

            def body():
                k0 = 0
                for ci, ck in enumerate(CHUNKS):
                    chunk_body(ci, ck, k0)
                    k0 += ck

            if REPEAT > 1:
                with tc.For_i(0, REPEAT, 1):
                    body()
            else:
                body()

    nc.compile()
    return nc


def get_nc():
    if "nc" not in _CACHE:
        _CACHE["nc"] = _build_nc()
    return _CACHE["nc"]


def _np_in_dt():
    import ml_dtypes

    return {"float32": np.float32, "float16": np.float16}.get(
        IN_DT, np.dtype(ml_dtypes.bfloat16)
    )


def make_in_maps(mixture_w, est_mask, W):
    dt = _np_in_dt()
    mixture_w = np.asarray(mixture_w).astype(dt)
    mask_q = np.clip(
        np.rint(np.asarray(est_mask, dtype=np.float32) * 255.0), 0, 255
    ).astype(np.uint8)
    wt = np.ascontiguousarray((np.asarray(W, dtype=np.float32).T / 255.0).astype(dt))
    in_maps = []
    for m in range(M):
        for kh in range(2):
            s0 = kh * KH
            in_maps.append(
                {
                    "mw": np.ascontiguousarray(mixture_w[m, :, s0 : s0 + KH]),
                    "mask": np.ascontiguousarray(
                        mask_q[m, :, :, s0 : s0 + KH]
                    ).reshape(C * N, KH),
                    "w": wt,
                }
            )
    return in_maps


def stitch(results):
    """results: list of 8 per-core outputs [C, HOP, QH] in (m, kh) order."""
    out = np.zeros((M, C, T), np.float32)
    for m in range(M):
        r0 = results[2 * m].transpose(0, 2, 1).reshape(C, TH)
        r1 = results[2 * m + 1].transpose(0, 2, 1).reshape(C, TH)
        out[m, :, :TH] = r0
        out[m, :, KH * HOP :] += r1
    return out


def kernel(mixture_w, est_mask, W):
    from concourse.bass_utils import run_bass_kernel_spmd

    nc = get_nc()
    in_maps = make_in_maps(mixture_w, est_mask, W)
    res = run_bass_kernel_spmd(nc, in_maps, list(range(M * 2)))
    return stitch([r["out"] for r in res.results])
